# revision 52
# baseline (speedup 1.0000x reference)
"""Trainium2 Bass kernel for causal multi-head attention.

Shapes (hardcoded): B=4, T=2048, D=1024, H=16, Dh=64, fp32 I/O.

Strategy (8 NeuronCores, tensor-parallel over heads):
  - Each core c owns heads (2c, 2c+1): Q^T/K^T projections for its 128
    head-dims over [B*T, D] (fp8e4m3 DoubleRow matmuls, contraction 256/step),
    V projection in fp16 (tokens-on-partitions layout for the AV stationary),
    then causal flash-style attention in scores-transposed orientation
    (S^T[k, q] blocks):
      * both heads' scores per key-block go into one [128, 2, 512] PSUM pair
        tile -> ONE exp per key-block on ScalarE (halves ACT instruction and
        semaphore-wait count; no max subtraction: logits are O(+-3))
      * diagonal blocks emit only the unmasked column range (trim), with
        [128,128] triangle mask multiplies
      * denominator via ones-columns embedded in the V stationary
      * division folded into PSUM->SBUF via a PE-broadcast reciprocal row
  - Per-batch AllToAll re-shards ctx^T from head-sharded to row-sharded
    (batch 3 in two halves so the final exchange is small); out-proj runs on
    128-token groups as PE gap-filler.
  - Projection chunks are woven between attention blocks one half-batch
    ahead, so every batch's attention (including the last) has PE filler and
    the PE never idles waiting on ScalarE exp.

All attention matmul operands are fp16; QK projections fp8e4m3 (DoubleRow);
accumulation fp32 in PSUM.
"""

import sys
from collections import deque

sys.path.insert(0, "/opt/trn_rl_repo")

import numpy as np

import concourse.bass as bass
import concourse.mybir as mybir
import concourse.tile as tile
from concourse import bacc
from concourse import bass_utils

N_CORES = 8
B, T, D, H, DH = 4, 2048, 1024, 16, 64
BT = B * T  # 8192
KS = D // 128  # 8 fp16 contraction subtiles
KS2 = D // 256  # 4 fp8 DoubleRow contraction subtiles
TC = 512  # t-chunk for projections
NTC = BT // TC  # 16
QC = 512  # query chunk in attention
NQC = T // QC  # 4 per batch
KB = 128  # key block
NKB = T // KB  # 16 per batch

F16 = mybir.dt.float16
F32 = mybir.dt.float32
F8 = mybir.dt.float8e4
DR = mybir.MatmulPerfMode.DoubleRow
EXP = mybir.ActivationFunctionType.Exp
MULT = mybir.AluOpType.mult
ADD = mybir.AluOpType.add

FP8_QK = True

_CACHE = {}


def _build(fp8_qk=FP8_QK):
    nc = bacc.Bacc("TRN2", target_bir_lowering=False, num_devices=N_CORES)

    if fp8_qk:
        x8_d = nc.dram_tensor("x8", [128, KS2, 2, BT], F8, kind="ExternalInput")
        wq_d = nc.dram_tensor("wq", [128, KS2, 2, 128], F8, kind="ExternalInput")
        wk_d = nc.dram_tensor("wk", [128, KS2, 2, 128], F8, kind="ExternalInput")
    else:
        wq_d = nc.dram_tensor("wq", [D, 128], F16, kind="ExternalInput")
        wk_d = nc.dram_tensor("wk", [D, 128], F16, kind="ExternalInput")
    x_d = nc.dram_tensor("x", [D, BT], F16, kind="ExternalInput")  # pre-transposed
    wv_d = nc.dram_tensor("wv", [D, 128], F16, kind="ExternalInput")
    wo_d = nc.dram_tensor("wo", [D, D], F16, kind="ExternalInput")
    bo_d = nc.dram_tensor("bo", [D], F32, kind="ExternalInput")
    tri_d = nc.dram_tensor("tri", [128, 128], F16, kind="ExternalInput")
    out_d = nc.dram_tensor("out", [B, 256, D], F32, kind="ExternalOutput")

    with tile.TileContext(nc) as tc:
        with (
            tc.tile_pool(name="persist", bufs=1) as persist,
            tc.tile_pool(name="x8p", bufs=4) as x8p,
            tc.tile_pool(name="x16p", bufs=4) as x16p,
            tc.tile_pool(name="work", bufs=4) as work,
            tc.tile_pool(name="tailp", bufs=2) as tailp,
            tc.tile_pool(name="ctxp", bufs=3) as ctxp,
            tc.tile_pool(name="outp", bufs=3) as outp,
            tc.tile_pool(name="ps", bufs=1, space="PSUM") as ps,
            tc.tile_pool(name="dram", bufs=1, space="DRAM") as dram,
        ):
            # ---- persistent weights / state ----
            if fp8_qk:
                wq_sb = persist.tile([128, KS2, 2, 128], F8)
                wk_sb = persist.tile([128, KS2, 2, 128], F8)
                nc.sync.dma_start(wq_sb[:], wq_d[:])
                nc.sync.dma_start(wk_sb[:], wk_d[:])
            else:
                wq_sb = persist.tile([128, KS, 128], F16)
                wk_sb = persist.tile([128, KS, 128], F16)
                nc.sync.dma_start(wq_sb[:], wq_d.rearrange("(o p) h -> p o h", p=128))
                nc.sync.dma_start(wk_sb[:], wk_d.rearrange("(o p) h -> p o h", p=128))
            wv_sb = persist.tile([128, KS, 128], F16)
            nc.sync.dma_start(wv_sb[:], wv_d.rearrange("(o p) h -> p o h", p=128))

            qt_sb = persist.tile([128, BT], F16)  # [2 heads x 64 dims, global t]
            kt_sb = persist.tile([128, BT], F16)
            # V layout per key-block kbg: per head a 65-col group [V 64 |
            # ones], so av[0:64] = ctx dims (partition range [0,64), aligned)
            # and av[64] = softmax denominator (partition base 64, aligned).
            v_sb = persist.tile([128, B * NKB, 130], F16)
            nc.vector.memset(v_sb[:, :, 64:65], 1.0)
            nc.vector.memset(v_sb[:, :, 129:130], 1.0)

            ones1 = persist.tile([1, 64], F16)
            nc.vector.memset(ones1[:], 1.0)
            tri_sb = persist.tile([128, 128], F16)
            nc.sync.dma_start(tri_sb[:], tri_d[:])
            ones_col = persist.tile([1, 128], F32)
            nc.vector.memset(ones_col[:], 1.0)
            bo_sb = persist.tile([1, D], F32)
            nc.sync.dma_start(bo_sb[:], bo_d[None, :])

            wo_sb = persist.tile([128, KS, D], F16)
            bias_sb = persist.tile([128, D], F32)
            ao_sbs = [
                persist.tile([128, KS, 256], F16, name=f"ao{b}", tag=f"ao{b}")
                for b in range(B)
            ]

            # ---- projection chunk units (PE gap-filler quanta) ----
            def chunk_units(tcn):
                t0 = tcn * TC
                st = {}

                def u_dma():
                    if fp8_qk:
                        st["x8"] = x8p.tile([128, KS2, 2, TC], F8, tag="x8", name="xt8")
                        nc.sync.dma_start(st["x8"][:], x8_d[:, :, :, t0 : t0 + TC])
                    st["x16"] = x16p.tile([128, KS, TC], F16, tag="x16", name="xt16")
                    nc.sync.dma_start(
                        st["x16"][:],
                        x_d[:, t0 : t0 + TC].rearrange("(o p) t -> p o t", p=128),
                    )

                def mk_qk(w_sb, dst):
                    def u():
                        pp = ps.tile([128, TC], F32, tag="pp", bufs=2, name="pp")
                        if fp8_qk:
                            for k in range(KS2):
                                nc.tensor.matmul(
                                    pp[:], w_sb[:, k], st["x8"][:, k],
                                    start=(k == 0), stop=(k == KS2 - 1),
                                    perf_mode=DR,
                                )
                        else:
                            for k in range(KS):
                                nc.tensor.matmul(
                                    pp[:], w_sb[:, k], st["x16"][:, k],
                                    start=(k == 0), stop=(k == KS - 1),
                                )
                        nc.vector.tensor_copy(dst[:, t0 : t0 + TC], pp[:])
                    return u

                def u_v():
                    # One self-contained unit: a pool tile's lifetime must be
                    # a contiguous emission window (other units allocate from
                    # the same tag slot rotation).
                    vp = ps.tile([128, 4, 128], F32, tag="pp", bufs=2, name="vp")
                    for sub in range(4):
                        for k in range(KS):
                            nc.tensor.matmul(
                                vp[:, sub, :],
                                st["x16"][:, k, sub * 128 : (sub + 1) * 128],
                                wv_sb[:, k],
                                start=(k == 0), stop=(k == KS - 1),
                            )
                    g0 = tcn * 4
                    nc.vector.tensor_copy(v_sb[:, g0 : g0 + 4, 0:64], vp[:, :, 0:64])
                    nc.vector.tensor_copy(
                        v_sb[:, g0 : g0 + 4, 65:129], vp[:, :, 64:128]
                    )

                return [u_dma, mk_qk(wq_sb, qt_sb), mk_qk(wk_sb, kt_sb), u_v]

            fillers = deque()
            tailq = deque()  # fillers reserved for qc-tail chain latency
            pending_oproj = []

            def fill(n=1, tail=False):
                for _ in range(n):
                    if tail and tailq:
                        tailq.popleft()()
                    elif fillers:
                        fillers.popleft()()
                    elif len(tailq) > 2:  # keep a reserve for upcoming tails
                        tailq.popleft()()

            # ---- out-proj tasks (also used as fillers) ----
            def mk_oproj(b, c0, nch):
                def u():
                    ao = ao_sbs[b]
                    op = ps.tile([128, TC], F32, tag="pp", bufs=2, name="op")
                    for r in range(KS):
                        nc.tensor.matmul(
                            op[:],
                            ao[:, r, c0 : c0 + 128],
                            wo_sb[:, r, nch * 512 : (nch + 1) * 512],
                            start=(r == 0), stop=(r == KS - 1),
                        )
                    osb = outp.tile([128, 512], F32, tag="osb", name="osb")
                    nc.vector.tensor_tensor(
                        osb[:], op[:], bias_sb[:, nch * 512 : (nch + 1) * 512], ADD
                    )
                    nc.sync.dma_start(
                        out_d[b, c0 : c0 + 128, nch * 512 : (nch + 1) * 512], osb[:]
                    )
                return u

            # ---- per-batch collective buffers ----
            cc_full = [
                (
                    dram.tile([N_CORES, 128, 256], F16, tag=f"ci{b}", name=f"ci{b}"),
                    dram.tile([N_CORES, 128, 256], F16, tag=f"co{b}", name=f"co{b}"),
                )
                for b in range(B - 1)
            ]
            cc_half = (
                dram.tile([N_CORES, 128, 128], F16, tag="cih", name="cih"),
                dram.tile([N_CORES, 128, 128], F16, tag="coh", name="coh"),
            )
            cc_qtr = [
                (
                    dram.tile([N_CORES, 128, 64], F16, tag=f"ciq{q}", name=f"ciq{q}"),
                    dram.tile([N_CORES, 128, 64], F16, tag=f"coq{q}", name=f"coq{q}"),
                )
                for q in range(2)
            ]

            # ---- attention ----
            def emit_se(b, qc, kb):
                """Scores pair + ONE exp (+ triangle masks on the diagonal
                128-square) for both heads of key-block kb. Diagonal blocks
                emit only columns [128*i:] (i = kb - 4*qc)."""
                i = kb - 4 * qc
                off = 128 * i if i >= 0 else 0
                q0 = b * T + qc * QC
                k0 = b * T + kb * KB
                sp = ps.tile([128, 2, QC], F32, tag="s", bufs=2, name="sp")
                for h in (0, 1):
                    hs = slice(h * 64, (h + 1) * 64)
                    nc.tensor.matmul(
                        sp[:, h, off:QC],
                        kt_sb[hs, k0 : k0 + KB],
                        qt_sb[hs, q0 + off : q0 + QC],
                    )
                e2t = work.tile([128, 2, QC], F16, tag="e", name="e2t")
                nc.scalar.activation(
                    e2t[:, :, off:QC], sp[:, :, off:QC], EXP, scale=0.125
                )
                if i >= 0:
                    nc.vector.tensor_tensor(
                        e2t[:, 0, off : off + 128], e2t[:, 0, off : off + 128],
                        tri_sb[:], MULT,
                    )
                    nc.vector.tensor_tensor(
                        e2t[:, 1, off : off + 128], e2t[:, 1, off : off + 128],
                        tri_sb[:], MULT,
                    )
                return (e2t, off)

            def mk_tail(b, qc, av0, av1):
                def tail():
                    fill(1, tail=True)
                    d2 = tailp.tile([1, 2 * QC], F32, tag="d2", name="d2")
                    nc.vector.tensor_copy(d2[:, 0:QC], av0[64:65, :])
                    nc.vector.tensor_copy(d2[:, QC : 2 * QC], av1[64:65, :])
                    r2 = tailp.tile([1, 2 * QC], F32, tag="r2", name="r2")
                    nc.vector.reciprocal_approx_fast(r2[:], d2[:])
                    r2h = tailp.tile([1, 2 * QC], F16, tag="r2h", name="r2h")
                    nc.vector.tensor_copy(r2h[:], r2[:])
                    fill(1, tail=True)
                    # broadcast 1/denom to 64 partitions per head via rank-1
                    # matmuls
                    rb = ps.tile([128, QC], F32, tag="s", bufs=2, name="rb")
                    nc.tensor.matmul(rb[0:64, :], ones1[:], r2h[:, 0:QC])
                    nc.tensor.matmul(rb[64:128, :], ones1[:], r2h[:, QC : 2 * QC])
                    rbsb = tailp.tile([128, QC], F32, tag="rbsb", name="rbsb")
                    nc.vector.tensor_copy(rbsb[:], rb[:])
                    ctx2 = ctxp.tile([128, QC], F16, tag="ctx", name="ctx")
                    nc.vector.tensor_tensor(
                        ctx2[0:64, :], av0[0:64, :], rbsb[0:64, :], MULT
                    )
                    nc.vector.tensor_tensor(
                        ctx2[64:128, :], av1[0:64, :], rbsb[64:128, :], MULT
                    )
                    if b < B - 1:
                        cin, cout = cc_full[b]
                        nc.sync.dma_start(
                            cin[2 * qc : 2 * qc + 2].rearrange("s p f -> p s f"),
                            ctx2[:].rearrange("p (s f) -> p s f", s=2),
                        )
                        if qc == NQC - 1:
                            nc.gpsimd.collective_compute(
                                "AllToAll",
                                mybir.AluOpType.bypass,
                                replica_groups=[list(range(N_CORES))],
                                ins=[cin[:]],
                                outs=[cout[:]],
                            )
                            nc.sync.dma_start(
                                ao_sbs[b][:], cout[:].rearrange("r p t -> p r t")
                            )
                            for j in range(2):
                                for nch in range(2):
                                    pending_oproj.append(mk_oproj(b, 128 * j, nch))
                    elif qc < 2:
                        # batch 3 first half: one [8,128,128] exchange
                        cin, cout = cc_half
                        nc.sync.dma_start(
                            cin[4 * qc : 4 * qc + 4].rearrange("s p f -> p s f"),
                            ctx2[:].rearrange("p (s f) -> p s f", s=4),
                        )
                        if qc == 1:
                            nc.gpsimd.collective_compute(
                                "AllToAll",
                                mybir.AluOpType.bypass,
                                replica_groups=[list(range(N_CORES))],
                                ins=[cin[:]],
                                outs=[cout[:]],
                            )
                            nc.sync.dma_start(
                                ao_sbs[b][:, :, 0:128],
                                cout[:].rearrange("r p t -> p r t"),
                            )
                            for nch in range(2):
                                pending_oproj.append(mk_oproj(b, 0, nch))
                    else:
                        # batch 3 second half: per-qc [8,128,64] quarters so
                        # the very last exchange (and PE stall) is minimal
                        cin, cout = cc_qtr[qc - 2]
                        nc.sync.dma_start(
                            cin[:].rearrange("s p f -> p s f"),
                            ctx2[:].rearrange("p (s f) -> p s f", s=8),
                        )
                        nc.gpsimd.collective_compute(
                            "AllToAll",
                            mybir.AluOpType.bypass,
                            replica_groups=[list(range(N_CORES))],
                            ins=[cin[:]],
                            outs=[cout[:]],
                        )
                        c0 = 128 + 64 * (qc - 2)
                        nc.sync.dma_start(
                            ao_sbs[b][:, :, c0 : c0 + 64],
                            cout[:].rearrange("r p t -> p r t"),
                        )
                        if qc == 3:
                            for nch in range(2):
                                pending_oproj.append(mk_oproj(b, 128, nch))
                return tail

            # ---- emission ----
            # tiny dummy AllToAll to absorb the cold-start cost of the CC
            # mesh (the first real collective otherwise takes 30-75us)
            warm_i = dram.tile([N_CORES, 1, 16], F16, tag="warm_i", name="warm_i")
            warm_o = dram.tile([N_CORES, 1, 16], F16, tag="warm_o", name="warm_o")
            nc.gpsimd.collective_compute(
                "AllToAll",
                mybir.AluOpType.bypass,
                replica_groups=[list(range(N_CORES))],
                ins=[warm_i[:]],
                outs=[warm_o[:]],
            )
            # batch 0 projections upfront (DMAs first so compute never waits)
            b0units = [chunk_units(tcn) for tcn in range(4)]
            for cu in b0units:
                cu[0]()
            for cu in b0units:
                for u in cu[1:]:
                    u()
            # deferred big weight DMA + bias broadcast (PE ones-trick); the
            # fp32 bias matmuls run after the PE has ramped out of its low
            # p-state
            nc.sync.dma_start(wo_sb[:], wo_d.rearrange("(r p) n -> p r n", p=128))

            def u_bias():
                for nchb in range(2):
                    bps = ps.tile([128, 512], F32, tag="pp", bufs=2, name="bps")
                    nc.tensor.matmul(
                        bps[:], ones_col[:], bo_sb[:, nchb * 512 : (nchb + 1) * 512]
                    )
                    nc.vector.tensor_copy(
                        bias_sb[:, nchb * 512 : (nchb + 1) * 512], bps[:]
                    )

            fillers.append(u_bias)

            prev_tail = None
            for b in range(B):
                # weave: batch b's own second-half chunks (drained during its
                # qc0-1, needed from qc2) + next batch's first half — so every
                # batch's attention, including the last, has PE filler. The
                # big V units go to the tail reserve (they cover the qc-tail
                # reciprocal-chain latency); flush leftovers first so no pool
                # tile's lifetime crosses a later same-tag allocation.
                while tailq:
                    fillers.append(tailq.popleft())
                push = []
                if b >= 1:
                    push += [4 * b + 2, 4 * b + 3]
                if b + 1 < B:
                    push += [4 * b + 4, 4 * b + 5]
                for tcn in push:
                    units = chunk_units(tcn)
                    units[0]()  # x-chunk DMAs issue now (no PE cost)
                    fillers.extend(units[1:-1])
                    tailq.append(units[-1])
                for qc in range(NQC):
                    nkb = 4 * qc + 4
                    if qc >= 2 and pending_oproj:
                        # out-proj for the exchange launched ~a batch ago —
                        # deferred so the PE stream never blocks on an
                        # in-flight AllToAll, and reserved for qc-tail fill
                        # points where the PE otherwise idles on the
                        # reciprocal chain
                        tailq.extend(pending_oproj)
                        pending_oproj.clear()
                    window = deque(emit_se(b, qc, k) for k in range(2))
                    if prev_tail is not None:
                        prev_tail()
                    av0 = ps.tile([65, QC], F32, tag="av", bufs=2, name="av0")
                    av1 = ps.tile([65, QC], F32, tag="av", bufs=2, name="av1")
                    for kb in range(nkb):
                        if kb + 2 < nkb:
                            window.append(emit_se(b, qc, kb + 2))
                        fill(1)
                        e2t, off = window.popleft()
                        kbg = b * NKB + kb
                        first, last = kb == 0, kb == nkb - 1
                        nc.tensor.matmul(
                            av0[:, off:QC], v_sb[:, kbg, 0:65], e2t[:, 0, off:QC],
                            start=first, stop=last,
                        )
                        nc.tensor.matmul(
                            av1[:, off:QC], v_sb[:, kbg, 65:130], e2t[:, 1, off:QC],
                            start=first, stop=last,
                        )
                    prev_tail = mk_tail(b, qc, av0, av1)
            tailq.extend(pending_oproj)
            pending_oproj.clear()
            prev_tail()
            fillers.extend(pending_oproj)
            fillers.extend(tailq)
            pending_oproj.clear()
            tailq.clear()
            while fillers:
                fillers.popleft()()

    nc.compile()
    return nc


def _get_nc():
    if "nc" not in _CACHE:
        _CACHE["nc"] = _build()
    return _CACHE["nc"]


def prepare_in_maps(x, Wq, Wk, Wv, Wo, bo):
    f8np = mybir.dt.np(F8)
    x32 = np.ascontiguousarray(np.asarray(x, dtype=np.float32).reshape(BT, D).T)
    x16 = x32.astype(np.float16)
    bo32 = np.ascontiguousarray(np.asarray(bo, dtype=np.float32))
    wo16 = np.asarray(Wo, dtype=np.float32).astype(np.float16)
    tri = (np.arange(128)[None, :] >= np.arange(128)[:, None]).astype(np.float16)

    in_maps = []
    if FP8_QK:
        x8 = np.ascontiguousarray(
            x32.reshape(KS2, 2, 128, BT).transpose(2, 0, 1, 3)
        ).astype(f8np)
    for c in range(N_CORES):
        cs = slice(128 * c, 128 * (c + 1))
        m = {
            "x": x16,
            "wv": np.ascontiguousarray(np.asarray(Wv, np.float32)[:, cs]).astype(
                np.float16
            ),
            "wo": wo16,
            "bo": bo32,
            "tri": tri,
        }
        if FP8_QK:
            m["x8"] = x8
            m["wq"] = np.ascontiguousarray(
                np.asarray(Wq, np.float32)[:, cs]
                .reshape(KS2, 2, 128, 128)
                .transpose(2, 0, 1, 3)
            ).astype(f8np)
            m["wk"] = np.ascontiguousarray(
                np.asarray(Wk, np.float32)[:, cs]
                .reshape(KS2, 2, 128, 128)
                .transpose(2, 0, 1, 3)
            ).astype(f8np)
        else:
            m["wq"] = np.ascontiguousarray(np.asarray(Wq, np.float32)[:, cs]).astype(
                np.float16
            )
            m["wk"] = np.ascontiguousarray(np.asarray(Wk, np.float32)[:, cs]).astype(
                np.float16
            )
        in_maps.append(m)
    return in_maps


def kernel(x, Wq, Wk, Wv, Wo, bo, _trace=False):
    nc = _get_nc()
    in_maps = prepare_in_maps(x, Wq, Wk, Wv, Wo, bo)
    res = bass_utils.run_bass_kernel_spmd(
        nc, in_maps, list(range(N_CORES)), trace=_trace
    )
    if _trace:
        _CACHE["last_results"] = res
    out = np.empty((B, T, D), dtype=np.float32)
    for c in range(N_CORES):
        oc = res.results[c]["out"]  # [B, 256, D]
        for b in range(B - 1):
            out[b, 256 * c : 256 * c + 256, :] = oc[b]
        # batch 3: rows 0:128 = first-half exchange (128-token shards),
        # rows 128:192 / 192:256 = qc2 / qc3 quarters (64-token shards)
        out[B - 1, 128 * c : 128 * c + 128, :] = oc[B - 1, 0:128]
        for q in range(2):
            r0 = 512 * (2 + q) + 64 * c
            out[B - 1, r0 : r0 + 64, :] = oc[B - 1, 128 + 64 * q : 192 + 64 * q]
    return out


# revision 53
# speedup vs baseline: 1.0597x; 1.0597x over previous
"""Trainium2 Bass kernel for causal multi-head attention.

Shapes (hardcoded): B=4, T=2048, D=1024, H=16, Dh=64, fp32 I/O.

Strategy (8 NeuronCores, tensor-parallel over heads):
  - Each core c owns heads (2c, 2c+1): Q^T/K^T projections for its 128
    head-dims over [B*T, D] (fp8e4m3 DoubleRow matmuls, contraction 256/step),
    V projection in fp16 (tokens-on-partitions layout for the AV stationary),
    then causal flash-style attention in scores-transposed orientation
    (S^T[k, q] blocks):
      * both heads' scores per key-block go into one [128, 2, 512] PSUM pair
        tile -> ONE exp per key-block on ScalarE (halves ACT instruction and
        semaphore-wait count; no max subtraction: logits are O(+-3))
      * diagonal blocks emit only the unmasked column range (trim), with
        [128,128] triangle mask multiplies
      * denominator via ones-columns embedded in the V stationary
      * division folded into PSUM->SBUF via a PE-broadcast reciprocal row
  - Per-batch AllToAll re-shards ctx^T from head-sharded to row-sharded
    (batch 3 in two halves so the final exchange is small); out-proj runs on
    128-token groups as PE gap-filler.
  - Projection chunks are woven between attention blocks one half-batch
    ahead, so every batch's attention (including the last) has PE filler and
    the PE never idles waiting on ScalarE exp.

All attention matmul operands are fp16; QK projections fp8e4m3 (DoubleRow);
accumulation fp32 in PSUM.
"""

import sys
from collections import deque

sys.path.insert(0, "/opt/trn_rl_repo")

import numpy as np

import concourse.bass as bass
import concourse.mybir as mybir
import concourse.tile as tile
from concourse import bacc
from concourse import bass_utils

N_CORES = 8
B, T, D, H, DH = 4, 2048, 1024, 16, 64
BT = B * T  # 8192
KS = D // 128  # 8 fp16 contraction subtiles
KS2 = D // 256  # 4 fp8 DoubleRow contraction subtiles
TC = 512  # t-chunk for projections
NTC = BT // TC  # 16
QC = 512  # query chunk in attention
NQC = T // QC  # 4 per batch
KB = 128  # key block
NKB = T // KB  # 16 per batch

F16 = mybir.dt.float16
F32 = mybir.dt.float32
F8 = mybir.dt.float8e4
DR = mybir.MatmulPerfMode.DoubleRow
EXP = mybir.ActivationFunctionType.Exp
MULT = mybir.AluOpType.mult
ADD = mybir.AluOpType.add

FP8_QK = True

_CACHE = {}


def _build(fp8_qk=FP8_QK):
    nc = bacc.Bacc("TRN2", target_bir_lowering=False, num_devices=N_CORES)

    if fp8_qk:
        x8_d = nc.dram_tensor("x8", [128, KS2, 2, BT], F8, kind="ExternalInput")
        wq_d = nc.dram_tensor("wq", [128, KS2, 2, 128], F8, kind="ExternalInput")
        wk_d = nc.dram_tensor("wk", [128, KS2, 2, 128], F8, kind="ExternalInput")
    else:
        wq_d = nc.dram_tensor("wq", [D, 128], F16, kind="ExternalInput")
        wk_d = nc.dram_tensor("wk", [D, 128], F16, kind="ExternalInput")
    x_d = nc.dram_tensor("x", [D, BT], F16, kind="ExternalInput")  # pre-transposed
    wv_d = nc.dram_tensor("wv", [D, 128], F16, kind="ExternalInput")
    wo_d = nc.dram_tensor("wo", [D, D], F16, kind="ExternalInput")
    bo_d = nc.dram_tensor("bo", [D], F32, kind="ExternalInput")
    tri_d = nc.dram_tensor("tri", [128, 128], F16, kind="ExternalInput")
    out_d = nc.dram_tensor("out", [B, 256, D], F32, kind="ExternalOutput")

    with tile.TileContext(nc) as tc:
        with (
            tc.tile_pool(name="persist", bufs=1) as persist,
            tc.tile_pool(name="x8p", bufs=4) as x8p,
            tc.tile_pool(name="x16p", bufs=4) as x16p,
            tc.tile_pool(name="work", bufs=4) as work,
            tc.tile_pool(name="tailp", bufs=2) as tailp,
            tc.tile_pool(name="ctxp", bufs=3) as ctxp,
            tc.tile_pool(name="outp", bufs=3) as outp,
            tc.tile_pool(name="ps", bufs=1, space="PSUM") as ps,
            tc.tile_pool(name="dram", bufs=1, space="DRAM") as dram,
        ):
            # ---- persistent weights / state ----
            if fp8_qk:
                wq_sb = persist.tile([128, KS2, 2, 128], F8)
                wk_sb = persist.tile([128, KS2, 2, 128], F8)
                nc.sync.dma_start(wq_sb[:], wq_d[:])
                nc.sync.dma_start(wk_sb[:], wk_d[:])
            else:
                wq_sb = persist.tile([128, KS, 128], F16)
                wk_sb = persist.tile([128, KS, 128], F16)
                nc.sync.dma_start(wq_sb[:], wq_d.rearrange("(o p) h -> p o h", p=128))
                nc.sync.dma_start(wk_sb[:], wk_d.rearrange("(o p) h -> p o h", p=128))
            wv_sb = persist.tile([128, KS, 128], F16)
            nc.sync.dma_start(wv_sb[:], wv_d.rearrange("(o p) h -> p o h", p=128))

            qt_sb = persist.tile([128, BT], F16)  # [2 heads x 64 dims, global t]
            kt_sb = persist.tile([128, BT], F16)
            # V layout per key-block kbg: per head a 65-col group [V 64 |
            # ones], so av[0:64] = ctx dims (partition range [0,64), aligned)
            # and av[64] = softmax denominator (partition base 64, aligned).
            v_sb = persist.tile([128, B * NKB, 130], F16)
            nc.vector.memset(v_sb[:, :, 64:65], 1.0)
            nc.vector.memset(v_sb[:, :, 129:130], 1.0)

            ones1 = persist.tile([1, 64], F16)
            nc.vector.memset(ones1[:], 1.0)
            tri_sb = persist.tile([128, 128], F16)
            nc.sync.dma_start(tri_sb[:], tri_d[:])
            ones_col = persist.tile([1, 128], F32)
            nc.vector.memset(ones_col[:], 1.0)
            bo_sb = persist.tile([1, D], F32)
            nc.sync.dma_start(bo_sb[:], bo_d[None, :])

            wo_sb = persist.tile([128, KS, D], F16)
            bias_sb = persist.tile([128, D], F32)
            ao_sbs = [
                persist.tile([128, KS, 256], F16, name=f"ao{b}", tag=f"ao{b}")
                for b in range(B)
            ]

            # ---- projection chunk units (PE gap-filler quanta) ----
            def chunk_units(tcn):
                t0 = tcn * TC
                st = {}

                def u_dma():
                    if fp8_qk:
                        st["x8"] = x8p.tile([128, KS2, 2, TC], F8, tag="x8", name="xt8")
                        nc.sync.dma_start(st["x8"][:], x8_d[:, :, :, t0 : t0 + TC])
                    st["x16"] = x16p.tile([128, KS, TC], F16, tag="x16", name="xt16")
                    nc.sync.dma_start(
                        st["x16"][:],
                        x_d[:, t0 : t0 + TC].rearrange("(o p) t -> p o t", p=128),
                    )

                def mk_qk(w_sb, dst):
                    def u():
                        pp = ps.tile([128, TC], F32, tag="pp", bufs=2, name="pp")
                        if fp8_qk:
                            for k in range(KS2):
                                nc.tensor.matmul(
                                    pp[:], w_sb[:, k], st["x8"][:, k],
                                    start=(k == 0), stop=(k == KS2 - 1),
                                    perf_mode=DR,
                                )
                        else:
                            for k in range(KS):
                                nc.tensor.matmul(
                                    pp[:], w_sb[:, k], st["x16"][:, k],
                                    start=(k == 0), stop=(k == KS - 1),
                                )
                        nc.vector.tensor_copy(dst[:, t0 : t0 + TC], pp[:])
                    return u

                def u_v():
                    # One self-contained unit: a pool tile's lifetime must be
                    # a contiguous emission window (other units allocate from
                    # the same tag slot rotation).
                    vp = ps.tile([128, 4, 128], F32, tag="pp", bufs=2, name="vp")
                    for sub in range(4):
                        for k in range(KS):
                            nc.tensor.matmul(
                                vp[:, sub, :],
                                st["x16"][:, k, sub * 128 : (sub + 1) * 128],
                                wv_sb[:, k],
                                start=(k == 0), stop=(k == KS - 1),
                            )
                    g0 = tcn * 4
                    nc.vector.tensor_copy(v_sb[:, g0 : g0 + 4, 0:64], vp[:, :, 0:64])
                    nc.vector.tensor_copy(
                        v_sb[:, g0 : g0 + 4, 65:129], vp[:, :, 64:128]
                    )

                return [u_dma, mk_qk(wq_sb, qt_sb), mk_qk(wk_sb, kt_sb), u_v]

            fillers = deque()
            tailq = deque()  # fillers reserved for qc-tail chain latency
            pending_oproj = []

            def fill(n=1, tail=False):
                for _ in range(n):
                    if tail and tailq:
                        tailq.popleft()()
                    elif fillers:
                        fillers.popleft()()
                    elif len(tailq) > 2:  # keep a reserve for upcoming tails
                        tailq.popleft()()

            # ---- out-proj tasks (also used as fillers) ----
            def mk_oproj(b, c0, nch):
                def u():
                    ao = ao_sbs[b]
                    op = ps.tile([128, TC], F32, tag="pp", bufs=2, name="op")
                    for r in range(KS):
                        nc.tensor.matmul(
                            op[:],
                            ao[:, r, c0 : c0 + 128],
                            wo_sb[:, r, nch * 512 : (nch + 1) * 512],
                            start=(r == 0), stop=(r == KS - 1),
                        )
                    osb = outp.tile([128, 512], F32, tag="osb", name="osb")
                    nc.vector.tensor_tensor(
                        osb[:], op[:], bias_sb[:, nch * 512 : (nch + 1) * 512], ADD
                    )
                    nc.sync.dma_start(
                        out_d[b, c0 : c0 + 128, nch * 512 : (nch + 1) * 512], osb[:]
                    )
                return u

            # ---- per-batch collective buffers ----
            cc_full = [
                (
                    dram.tile([N_CORES, 128, 256], F16, tag=f"ci{b}", name=f"ci{b}"),
                    dram.tile([N_CORES, 128, 256], F16, tag=f"co{b}", name=f"co{b}"),
                )
                for b in range(B - 1)
            ]
            cc_half = (
                dram.tile([N_CORES, 128, 128], F16, tag="cih", name="cih"),
                dram.tile([N_CORES, 128, 128], F16, tag="coh", name="coh"),
            )
            cc_qtr = [
                (
                    dram.tile([N_CORES, 128, 64], F16, tag=f"ciq{q}", name=f"ciq{q}"),
                    dram.tile([N_CORES, 128, 64], F16, tag=f"coq{q}", name=f"coq{q}"),
                )
                for q in range(2)
            ]

            # ---- attention ----
            def emit_se(b, qc, kb):
                """Scores pair + ONE exp (+ triangle masks on the diagonal
                128-square) for both heads of key-block kb. Diagonal blocks
                emit only columns [128*i:] (i = kb - 4*qc)."""
                i = kb - 4 * qc
                off = 128 * i if i >= 0 else 0
                q0 = b * T + qc * QC
                k0 = b * T + kb * KB
                sp = ps.tile([128, 2, QC], F32, tag="s", bufs=2, name="sp")
                for h in (0, 1):
                    hs = slice(h * 64, (h + 1) * 64)
                    nc.tensor.matmul(
                        sp[:, h, off:QC],
                        kt_sb[hs, k0 : k0 + KB],
                        qt_sb[hs, q0 + off : q0 + QC],
                    )
                e2t = work.tile([128, 2, QC], F16, tag="e", name="e2t")
                nc.scalar.activation(
                    e2t[:, :, off:QC], sp[:, :, off:QC], EXP, scale=0.125
                )
                if i >= 0:
                    nc.vector.tensor_tensor(
                        e2t[:, 0, off : off + 128], e2t[:, 0, off : off + 128],
                        tri_sb[:], MULT,
                    )
                    nc.vector.tensor_tensor(
                        e2t[:, 1, off : off + 128], e2t[:, 1, off : off + 128],
                        tri_sb[:], MULT,
                    )
                return (e2t, off)

            def mk_tail(b, qc, av0, av1):
                def tail():
                    fill(1, tail=True)
                    d2 = tailp.tile([1, 2 * QC], F32, tag="d2", name="d2")
                    nc.vector.tensor_copy(d2[:, 0:QC], av0[64:65, :])
                    nc.vector.tensor_copy(d2[:, QC : 2 * QC], av1[64:65, :])
                    r2 = tailp.tile([1, 2 * QC], F32, tag="r2", name="r2")
                    nc.vector.reciprocal_approx_fast(r2[:], d2[:])
                    r2h = tailp.tile([1, 2 * QC], F16, tag="r2h", name="r2h")
                    nc.vector.tensor_copy(r2h[:], r2[:])
                    fill(1, tail=True)
                    # broadcast 1/denom to 64 partitions per head via rank-1
                    # matmuls
                    rb = ps.tile([128, QC], F32, tag="s", bufs=2, name="rb")
                    nc.tensor.matmul(rb[0:64, :], ones1[:], r2h[:, 0:QC])
                    nc.tensor.matmul(rb[64:128, :], ones1[:], r2h[:, QC : 2 * QC])
                    rbsb = tailp.tile([128, QC], F32, tag="rbsb", name="rbsb")
                    nc.vector.tensor_copy(rbsb[:], rb[:])
                    ctx2 = ctxp.tile([128, QC], F16, tag="ctx", name="ctx")
                    nc.vector.tensor_tensor(
                        ctx2[0:64, :], av0[0:64, :], rbsb[0:64, :], MULT
                    )
                    nc.vector.tensor_tensor(
                        ctx2[64:128, :], av1[0:64, :], rbsb[64:128, :], MULT
                    )
                    if b < B - 1:
                        cin, cout = cc_full[b]
                        nc.sync.dma_start(
                            cin[2 * qc : 2 * qc + 2].rearrange("s p f -> p s f"),
                            ctx2[:].rearrange("p (s f) -> p s f", s=2),
                        )
                        if qc == NQC - 1:
                            nc.gpsimd.collective_compute(
                                "AllToAll",
                                mybir.AluOpType.bypass,
                                replica_groups=[list(range(N_CORES))],
                                ins=[cin[:]],
                                outs=[cout[:]],
                            )
                            nc.sync.dma_start(
                                ao_sbs[b][:], cout[:].rearrange("r p t -> p r t")
                            )
                            for j in range(2):
                                for nch in range(2):
                                    pending_oproj.append(mk_oproj(b, 128 * j, nch))
                    elif qc < 2:
                        # batch 3 first half: one [8,128,128] exchange
                        cin, cout = cc_half
                        nc.sync.dma_start(
                            cin[4 * qc : 4 * qc + 4].rearrange("s p f -> p s f"),
                            ctx2[:].rearrange("p (s f) -> p s f", s=4),
                        )
                        if qc == 1:
                            nc.gpsimd.collective_compute(
                                "AllToAll",
                                mybir.AluOpType.bypass,
                                replica_groups=[list(range(N_CORES))],
                                ins=[cin[:]],
                                outs=[cout[:]],
                            )
                            nc.sync.dma_start(
                                ao_sbs[b][:, :, 0:128],
                                cout[:].rearrange("r p t -> p r t"),
                            )
                            for nch in range(2):
                                pending_oproj.append(mk_oproj(b, 0, nch))
                    else:
                        # batch 3 second half: per-qc [8,128,64] quarters so
                        # the very last exchange (and PE stall) is minimal
                        cin, cout = cc_qtr[qc - 2]
                        nc.sync.dma_start(
                            cin[:].rearrange("s p f -> p s f"),
                            ctx2[:].rearrange("p (s f) -> p s f", s=8),
                        )
                        nc.gpsimd.collective_compute(
                            "AllToAll",
                            mybir.AluOpType.bypass,
                            replica_groups=[list(range(N_CORES))],
                            ins=[cin[:]],
                            outs=[cout[:]],
                        )
                        c0 = 128 + 64 * (qc - 2)
                        nc.sync.dma_start(
                            ao_sbs[b][:, :, c0 : c0 + 64],
                            cout[:].rearrange("r p t -> p r t"),
                        )
                        if qc == 3:
                            for nch in range(2):
                                pending_oproj.append(mk_oproj(b, 128, nch))
                return tail

            # ---- emission ----
            # tiny dummy AllToAll to absorb the cold-start cost of the CC
            # mesh (the first real collective otherwise takes 30-75us)
            warm_i = dram.tile([N_CORES, 1, 16], F16, tag="warm_i", name="warm_i")
            warm_o = dram.tile([N_CORES, 1, 16], F16, tag="warm_o", name="warm_o")
            nc.gpsimd.collective_compute(
                "AllToAll",
                mybir.AluOpType.bypass,
                replica_groups=[list(range(N_CORES))],
                ins=[warm_i[:]],
                outs=[warm_o[:]],
            )
            # batch 0 projections upfront (DMAs first so compute never waits)
            b0units = [chunk_units(tcn) for tcn in range(4)]
            for cu in b0units:
                cu[0]()
            for cu in b0units:
                for u in cu[1:]:
                    u()
            # deferred big weight DMA + bias broadcast (PE ones-trick); the
            # fp32 bias matmuls run after the PE has ramped out of its low
            # p-state
            nc.sync.dma_start(wo_sb[:], wo_d.rearrange("(r p) n -> p r n", p=128))

            def u_bias():
                for nchb in range(2):
                    bps = ps.tile([128, 512], F32, tag="pp", bufs=2, name="bps")
                    nc.tensor.matmul(
                        bps[:], ones_col[:], bo_sb[:, nchb * 512 : (nchb + 1) * 512]
                    )
                    nc.vector.tensor_copy(
                        bias_sb[:, nchb * 512 : (nchb + 1) * 512], bps[:]
                    )

            fillers.append(u_bias)

            prev_tail = None
            for b in range(B):
                # weave: batch b's own second-half chunks (drained during its
                # qc0-1, needed from qc2) + next batch's first half — so every
                # batch's attention, including the last, has PE filler. The
                # big V units go to the tail reserve (they cover the qc-tail
                # reciprocal-chain latency); flush leftovers first so no pool
                # tile's lifetime crosses a later same-tag allocation.
                while tailq:
                    fillers.append(tailq.popleft())
                push = []
                if b >= 1:
                    push += [4 * b + 2, 4 * b + 3]
                if b + 1 < B:
                    push += [4 * b + 4, 4 * b + 5]
                for tcn in push:
                    units = chunk_units(tcn)
                    units[0]()  # x-chunk DMAs issue now (no PE cost)
                    fillers.extend(units[1:-1])
                    tailq.append(units[-1])
                for qc in range(NQC):
                    nkb = 4 * qc + 4
                    if qc >= 3 and pending_oproj:
                        # out-proj for the exchange launched ~a batch ago —
                        # deferred so the PE stream never blocks on an
                        # in-flight AllToAll, and reserved for qc-tail fill
                        # points where the PE otherwise idles on the
                        # reciprocal chain
                        tailq.extend(pending_oproj)
                        pending_oproj.clear()
                    window = deque(emit_se(b, qc, k) for k in range(2))
                    if prev_tail is not None:
                        prev_tail()
                    av0 = ps.tile([65, QC], F32, tag="av", bufs=2, name="av0")
                    av1 = ps.tile([65, QC], F32, tag="av", bufs=2, name="av1")
                    for kb in range(nkb):
                        if kb + 2 < nkb:
                            window.append(emit_se(b, qc, kb + 2))
                        fill(1)
                        e2t, off = window.popleft()
                        kbg = b * NKB + kb
                        first, last = kb == 0, kb == nkb - 1
                        nc.tensor.matmul(
                            av0[:, off:QC], v_sb[:, kbg, 0:65], e2t[:, 0, off:QC],
                            start=first, stop=last,
                        )
                        nc.tensor.matmul(
                            av1[:, off:QC], v_sb[:, kbg, 65:130], e2t[:, 1, off:QC],
                            start=first, stop=last,
                        )
                    prev_tail = mk_tail(b, qc, av0, av1)
            tailq.extend(pending_oproj)
            pending_oproj.clear()
            prev_tail()
            fillers.extend(pending_oproj)
            fillers.extend(tailq)
            pending_oproj.clear()
            tailq.clear()
            while fillers:
                fillers.popleft()()

    nc.compile()
    return nc


def _get_nc():
    if "nc" not in _CACHE:
        _CACHE["nc"] = _build()
    return _CACHE["nc"]


def prepare_in_maps(x, Wq, Wk, Wv, Wo, bo):
    f8np = mybir.dt.np(F8)
    x32 = np.ascontiguousarray(np.asarray(x, dtype=np.float32).reshape(BT, D).T)
    x16 = x32.astype(np.float16)
    bo32 = np.ascontiguousarray(np.asarray(bo, dtype=np.float32))
    wo16 = np.asarray(Wo, dtype=np.float32).astype(np.float16)
    tri = (np.arange(128)[None, :] >= np.arange(128)[:, None]).astype(np.float16)

    in_maps = []
    if FP8_QK:
        x8 = np.ascontiguousarray(
            x32.reshape(KS2, 2, 128, BT).transpose(2, 0, 1, 3)
        ).astype(f8np)
    for c in range(N_CORES):
        cs = slice(128 * c, 128 * (c + 1))
        m = {
            "x": x16,
            "wv": np.ascontiguousarray(np.asarray(Wv, np.float32)[:, cs]).astype(
                np.float16
            ),
            "wo": wo16,
            "bo": bo32,
            "tri": tri,
        }
        if FP8_QK:
            m["x8"] = x8
            m["wq"] = np.ascontiguousarray(
                np.asarray(Wq, np.float32)[:, cs]
                .reshape(KS2, 2, 128, 128)
                .transpose(2, 0, 1, 3)
            ).astype(f8np)
            m["wk"] = np.ascontiguousarray(
                np.asarray(Wk, np.float32)[:, cs]
                .reshape(KS2, 2, 128, 128)
                .transpose(2, 0, 1, 3)
            ).astype(f8np)
        else:
            m["wq"] = np.ascontiguousarray(np.asarray(Wq, np.float32)[:, cs]).astype(
                np.float16
            )
            m["wk"] = np.ascontiguousarray(np.asarray(Wk, np.float32)[:, cs]).astype(
                np.float16
            )
        in_maps.append(m)
    return in_maps


def kernel(x, Wq, Wk, Wv, Wo, bo, _trace=False):
    nc = _get_nc()
    in_maps = prepare_in_maps(x, Wq, Wk, Wv, Wo, bo)
    res = bass_utils.run_bass_kernel_spmd(
        nc, in_maps, list(range(N_CORES)), trace=_trace
    )
    if _trace:
        _CACHE["last_results"] = res
    out = np.empty((B, T, D), dtype=np.float32)
    for c in range(N_CORES):
        oc = res.results[c]["out"]  # [B, 256, D]
        for b in range(B - 1):
            out[b, 256 * c : 256 * c + 256, :] = oc[b]
        # batch 3: rows 0:128 = first-half exchange (128-token shards),
        # rows 128:192 / 192:256 = qc2 / qc3 quarters (64-token shards)
        out[B - 1, 128 * c : 128 * c + 128, :] = oc[B - 1, 0:128]
        for q in range(2):
            r0 = 512 * (2 + q) + 64 * c
            out[B - 1, r0 : r0 + 64, :] = oc[B - 1, 128 + 64 * q : 192 + 64 * q]
    return out
